# Initial kernel scaffold
#
"""Single-head causal attention (B=4, T=4096, n_embd=1024, head=64) on 8 trn2 cores.

One SPMD program, 8 cores, one launch.  Core c -> batch b=c//2, half h=c%2.
Causal-balanced q-block (512 rows) assignment: half0 {0,3,4,7}, half1 {1,2,5,6}.

To keep the instruction stream identical across cores, each core runs 4 fixed
attention "slots" with k-ranges {8,16,24,32} k-blocks (128 keys each).  A slot
hosts one of the core's q-blocks (which one is per-core DATA, not control flow):
  half0: slots host qb {0,3,4,7} (own nk {4,16,20,32})
  half1: slots host qb {1,2,5,6} (own nk {8,12,24,28})
The last 8 k-blocks of every slot get a mask multiply; the mask tile is selected
at runtime from a 6-pattern table (tri0..3, zero, ones) via dynamic-AP offsets
loaded from a per-core int32 vector.  This zeroes both the causal diagonal and
the slot padding (own nk < slot nk).

Math (S^T formulation, all fp32):
  S^T[tk,tq] = K_blk^T.T @ Q^T          (PE, psum [128, 2*512])
  P^T = exp(S^T / 8)                    (one ACT op over both banks; no max-sub
                                         needed: S ~ N(0,1), exp can't overflow)
  P^T *= mask (last 8 kbs of slot)      (DVE, dynamic-AP pattern select)
  O_aug^T[65,512] += V_aug_blk.T @ P^T  (PE; V_aug col 64 = ones => row 64 of
                                         O_aug accumulates the softmax denom)
Epilogue per slot: PE-transpose O_aug^T -> [128tq, 65], reciprocal of col 64,
ACT copy*scale -> natural [128,64] rows, DMA out.  Host reassembles slots.
"""

import numpy as np

B, T, NE, HD = 4, 4096, 1024, 64
QB = 512            # q-block width
KB = 128            # k-block width
NQB = T // QB       # 8 t-blocks
NT = NE // 128      # 8 n-tiles (projection contraction)
SLOT_NK = [8, 16, 24, 32]          # k-blocks per slot (pairs: 4, 8, 12, 16)
HALF_QBS = [[0, 3, 4, 7], [1, 2, 5, 6]]   # slot si hosts q-block HALF_QBS[h][si]
DUMP = 4 * QB        # dump column in qt_sel for unused panels (not used now)

_CACHE = {}


def _host_tables(half):
    """Per-core mask thresholds [32] and q-select offsets [4].

    Mask for slot si, masked-index j (k-block kx = SLOT_NK[si]-8+j):
    valid(i, c) iff qoff + c >= kx*128 + i  iff  (c - i) >= 128*kx - qoff.
    """
    thr = np.zeros(32, dtype=np.float32)
    qoffs = np.zeros(4, dtype=np.int32)
    for si, nk in enumerate(SLOT_NK):
        own_qb = HALF_QBS[half][si]
        qoffs[si] = own_qb * QB
        for j in range(8):
            kx = nk - 8 + j
            thr[si * 8 + j] = 128.0 * kx - float(qoffs[si])
    return thr, qoffs


def _build_program():
    import concourse.bass as bass
    import concourse.mybir as mybir
    import concourse.tile as tile

    f32 = mybir.dt.float32
    i32 = mybir.dt.int32
    AF = mybir.ActivationFunctionType
    MS = bass.MemorySpace
    nc = bass.Bass("TRN2", target_bir_lowering=True, debug=False,
                   enable_asserts=False)

    xt_d = nc.dram_tensor("xt", [NE, T], f32, kind="ExternalInput").ap()
    wkv_d = nc.dram_tensor("wkv", [NE, 128], f32, kind="ExternalInput").ap()
    wq_d = nc.dram_tensor("wq", [NE, HD], f32, kind="ExternalInput").ap()
    ident_d = nc.dram_tensor("ident", [128, 128], f32, kind="ExternalInput").ap()
    identh_d = nc.dram_tensor("identh", [128, 64], f32, kind="ExternalInput").ap()
    dtab_d = nc.dram_tensor("dtab", [128, QB], f32, kind="ExternalInput").ap()
    thr_d = nc.dram_tensor("thr", [128, 32], f32, kind="ExternalInput").ap()
    qoffs_d = nc.dram_tensor("qoffs", [1, 4], i32, kind="ExternalInput").ap()
    out_d = nc.dram_tensor("out", [4 * QB, HD], f32, kind="ExternalOutput").ap()

    with tile.TileContext(nc) as tc:
        with (
            tc.tile_pool(name="consts", bufs=1) as cpool,
            tc.tile_pool(name="big", bufs=1) as bigpool,
            tc.tile_pool(name="xt", bufs=2) as xtpool,
            tc.tile_pool(name="pt", bufs=3) as ptpool,
            tc.tile_pool(name="osb", bufs=2) as osbpool,
            tc.tile_pool(name="onat", bufs=2) as onatpool,
            tc.tile_pool(name="rec", bufs=2) as recpool,
            tc.tile_pool(name="sps", bufs=2, space=MS.PSUM) as spool,
            tc.tile_pool(name="ops", bufs=2, space=MS.PSUM) as opool,
            tc.tile_pool(name="projps", bufs=2, space=MS.PSUM) as projpool,
        ):
            # ---- constants ----
            wkv_sb = cpool.tile([128, NT, 128], f32)
            nc.gpsimd.dma_start(wkv_sb[:], wkv_d.rearrange("(nt p) m -> p nt m", p=128))
            wq_sb = cpool.tile([128, NT, HD], f32)
            nc.gpsimd.dma_start(wq_sb[:], wq_d.rearrange("(nt p) m -> p nt m", p=128))
            ident = cpool.tile([128, 128], f32)
            nc.gpsimd.dma_start(ident[:], ident_d[:])
            identh = cpool.tile([128, 64], f32)
            nc.gpsimd.dma_start(identh[:], identh_d[:])
            dtab = cpool.tile([128, QB], f32)
            nc.gpsimd.dma_start(dtab[:], dtab_d[:])
            thr = cpool.tile([128, 32], f32)
            nc.gpsimd.dma_start(thr[:], thr_d[:])
            qoffs = cpool.tile([1, 4], i32)
            nc.gpsimd.dma_start(qoffs[:], qoffs_d[:])

            # ---- persistent sbuf state ----
            kvt = bigpool.tile([128, T], f32)          # 0:64 K^T, 64:128 V^T
            qt_all = bigpool.tile([64, T], f32)        # Q^T all 8 panels
            qt_sel = bigpool.tile([64, 4 * QB], f32)   # slot-ordered Q^T
            v_aug = bigpool.tile([128, 32 * 65], f32)  # V natural + ones col
            nc.vector.memset(v_aug[:], 1.0)

            def dyn_load(ap, lo, hi):
                tmp = nc.vector.alloc_register(f"dyn{nc.next_id()}")
                nc.vector.reg_load(tmp, ap)
                return nc.vector.snap(tmp, donate=True, min_val=lo, max_val=hi)

            def emit_attention(si):
                nk = SLOT_NK[si]
                npair = nk // 2
                o_ps = opool.tile([65, QB], f32, tag="ops")
                for p in range(npair):
                    ka, kb2 = 2 * p, 2 * p + 1
                    s_ps = spool.tile([128, 2 * QB], f32, tag="sps")
                    nc.tensor.matmul(
                        s_ps[:, 0:QB],
                        kvt[0:64, ka * KB:(ka + 1) * KB],
                        qt_sel[:, si * QB:(si + 1) * QB],
                        start=True, stop=True)
                    nc.tensor.matmul(
                        s_ps[:, QB:2 * QB],
                        kvt[0:64, kb2 * KB:(kb2 + 1) * KB],
                        qt_sel[:, si * QB:(si + 1) * QB],
                        start=True, stop=True)
                    pt = ptpool.tile([128, 2 * QB], f32, tag="pt")
                    nc.scalar.activation(pt[:], s_ps[:], AF.Exp,
                                         scale=float(HD) ** -0.5)
                    for half_i, kx in enumerate((ka, kb2)):
                        j = kx - (nk - 8)
                        if j >= 0:
                            # pt = (dtab >= thr) * pt ; thr = 128*kx - qoff
                            nc.vector.scalar_tensor_tensor(
                                pt[:, half_i * QB:(half_i + 1) * QB],
                                dtab[:],
                                thr[:, si * 8 + j: si * 8 + j + 1],
                                pt[:, half_i * QB:(half_i + 1) * QB],
                                mybir.AluOpType.is_ge,
                                mybir.AluOpType.mult)
                    nc.tensor.matmul(
                        o_ps[:], v_aug[:, ka * 65:ka * 65 + 65], pt[:, 0:QB],
                        start=(p == 0), stop=False, skip_group_check=True)
                    nc.tensor.matmul(
                        o_ps[:], v_aug[:, kb2 * 65:kb2 * 65 + 65],
                        pt[:, QB:2 * QB],
                        start=False, stop=(p == npair - 1),
                        skip_group_check=True)
                # epilogue
                ot_sb = osbpool.tile([65, QB], f32, tag="osb")
                nc.scalar.copy(ot_sb[:], o_ps[:])
                for u in range(QB // 128):
                    tp_ps = projpool.tile([128, QB], f32, tag="proj")
                    nc.tensor.transpose(
                        tp_ps[:, 0:65], ot_sb[:, u * 128:(u + 1) * 128],
                        ident[0:65, 0:65])
                    rec = recpool.tile([128, 1], f32, tag="rec")
                    nc.vector.reciprocal(rec[:], tp_ps[:, 64:65])
                    o_nat = onatpool.tile([128, HD], f32, tag="onat")
                    nc.scalar.activation(o_nat[:], tp_ps[:, 0:HD], AF.Copy,
                                         scale=rec[:])
                    nc.sync.dma_start(
                        out_d[si * QB + u * 128: si * QB + (u + 1) * 128, :],
                        o_nat[:])

            # ---- main pipeline over t-blocks ----
            for tb in range(NQB):
                xt_sb = xtpool.tile([128, NT, QB], f32, tag="xt")
                nc.gpsimd.dma_start(
                    xt_sb[:],
                    xt_d[:, tb * QB:(tb + 1) * QB].rearrange(
                        "(nt p) t -> p nt t", p=128))
                kv_ps = projpool.tile([128, QB], f32, tag="proj")
                for ni in range(NT):
                    nc.tensor.matmul(kv_ps[:], wkv_sb[:, ni, :], xt_sb[:, ni, :],
                                     start=(ni == 0), stop=(ni == NT - 1))
                nc.vector.tensor_copy(kvt[:, tb * QB:(tb + 1) * QB], kv_ps[:])
                q_ps = projpool.tile([64, QB], f32, tag="proj")
                for ni in range(NT):
                    nc.tensor.matmul(q_ps[:], wq_sb[:, ni, :], xt_sb[:, ni, :],
                                     start=(ni == 0), stop=(ni == NT - 1))
                nc.vector.tensor_copy(qt_all[:, tb * QB:(tb + 1) * QB], q_ps[:])
                for j in range(QB // KB):
                    kb = tb * (QB // KB) + j
                    tp_ps = projpool.tile([128, QB], f32, tag="proj")
                    nc.tensor.transpose(
                        tp_ps[:, 0:64], kvt[64:128, kb * KB:(kb + 1) * KB],
                        identh[64:128, 0:64])
                    nc.scalar.copy(v_aug[:, kb * 65:kb * 65 + 64], tp_ps[:, 0:64])
                if tb % 2 == 1:
                    si = tb // 2
                    with tc.tile_critical():
                        qoff = dyn_load(qoffs[0:1, si:si + 1], 0, T - QB)
                        nc.vector.tensor_copy(
                            qt_sel[:, si * QB:(si + 1) * QB],
                            qt_all[:, bass.ds(qoff, QB)])
                    emit_attention(si)

    _legalize_matmul_waits(nc)
    return nc


def _legalize_matmul_waits(nc):
    """walrus' LW template encodes at most one sync-wait; hoist extra waits
    from Matmult instructions onto a preceding PE NoOp (same queue, so
    ordering semantics are identical)."""
    import concourse.mybir as mybir

    for f in nc.m.functions:
        for bb in f.blocks:
            new_insts = []
            for inst in bb.instructions:
                si = inst.sync_info
                if (si is not None and si.on_wait and len(si.on_wait) >= 2):
                    for w in si.on_wait:
                        nop = mybir.InstNoOp(
                            name=nc.get_next_instruction_name(),
                            text_hint="wait_hoist", bass_nofuse=True)
                        nop.engine = inst.engine
                        nop.sync_info = mybir.SyncInfo(
                            on_wait=[w], on_update=[])
                        new_insts.append(nop)
                    inst.sync_info = mybir.SyncInfo(
                        on_wait=[], on_update=list(si.on_update or []))
                new_insts.append(inst)
            del bb.instructions[:]
            for i in new_insts:
                bb.instructions.append(i)


def _make_inputs(x, Wq, Wk, Wv):
    wkv = np.ascontiguousarray(np.concatenate([Wk, Wv], axis=1), dtype=np.float32)
    wq = np.ascontiguousarray(np.asarray(Wq, dtype=np.float32))
    ident = np.eye(128, dtype=np.float32)
    identh = np.zeros((128, 64), dtype=np.float32)
    identh[64:128, :] = np.eye(64, dtype=np.float32)
    col = np.arange(QB, dtype=np.float32)[None, :]
    row = np.arange(128, dtype=np.float32)[:, None]
    dtab = np.ascontiguousarray((col - row).astype(np.float32))  # [128, QB]

    in_maps = []
    for c in range(8):
        b, half = c // 2, c % 2
        thr, qoffs = _host_tables(half)
        thr_rep = np.ascontiguousarray(np.tile(thr[None, :], (128, 1)))
        xt = np.ascontiguousarray(np.asarray(x[b], dtype=np.float32).T)
        in_maps.append({
            "xt": xt, "wkv": wkv, "wq": wq, "ident": ident, "identh": identh,
            "dtab": dtab, "thr": thr_rep, "qoffs": qoffs.reshape(1, 4),
        })
    return in_maps


def kernel(x, Wq, Wk, Wv, _want_results=False, _trace=False):
    from concourse import bass_utils

    if "prog" not in _CACHE:
        _CACHE["prog"] = _build_program()
    nc = _CACHE["prog"]
    in_maps = _make_inputs(x, Wq, Wk, Wv)
    res = bass_utils.run_bass_kernel_spmd(nc, in_maps, core_ids=list(range(8)),
                                          trace=_trace)
    out = np.zeros((B, T, HD), dtype=np.float32)
    for c in range(8):
        b, half = c // 2, c % 2
        o = res.results[c]["out"]
        for si in range(4):
            qb = HALF_QBS[half][si]
            out[b, qb * QB:(qb + 1) * QB, :] = o[si * QB:(si + 1) * QB, :]
    if _want_results:
        return out, res
    return out



# revision 10
# speedup vs baseline: 8617.1705x; 8617.1705x over previous
"""Single-head causal attention (B=4, T=4096, n_embd=1024, head=64) on 8 trn2 cores.

One SPMD program, 8 cores, one launch.  Core c -> batch b=c//2, half h=c%2.
Causal-balanced q-block (512 rows) assignment: half0 {0,3,4,7}, half1 {1,2,5,6}.

To keep the instruction stream identical across cores, each core runs 4 fixed
attention "slots" with k-ranges {8,16,24,32} k-blocks (128 keys each).  A slot
hosts one of the core's q-blocks; the host passes per-core inputs so the
program itself is position-independent:
  - xq  [NE, 4*QB]: the core's own q-block columns of x^T, slot-ordered.
  - thr [128, 32]:  mask thresholds (causal diagonal + slot padding).
The last 8 k-blocks of every slot get a mask multiply on the DVE
((dtab >= thr) * p, dtab = col - row in fp16), which zeroes both the causal
upper triangle and the slot padding (own nk < slot nk).

Math (S^T formulation, fp32 PSUM, fp16 attention weights):
  S^T[tk,tq] = K_blk^T.T @ Q^T          (PE fp32r, psum [128, 2*512])
  P^T = exp(S^T / 8)  -> fp16           (ACT, one op per k-block pair; no
                                         max-subtraction: S ~ N(0,1))
  P^T *= mask (last 8 kbs of slot)      (DVE, 4x mode on fp16)
  O[tq,65] += P^T_chunk.T @ V_aug_blk   (PE fp16, natural orientation;
                                         V_aug col 64 = ones => col 64 of
                                         O accumulates the softmax denom)
Epilogue per slot: reciprocal of col 64 (DVE), ACT copy*scale -> [128,64]
rows, DMA out.  Host reassembles slots.

Engine budget per core (cost model): PE ~50us (proj 20 + S 17 + PV 9 + misc),
ACT ~46us (exp 41 + out scaling), DVE ~40us (copies + masks + recips),
Pool/SP ~30us each (input DMA split across both queues).
"""

import numpy as np

B, T, NE, HD = 4, 4096, 1024, 64
QB = 512            # q-block width
KB = 128            # k-block width
NQB = T // QB       # 8 t-blocks
NT = NE // 128      # 8 n-tiles (projection contraction)
SLOT_NK = [8, 16, 24, 32]          # k-blocks per slot (pairs: 4, 8, 12, 16)
HALF_QBS = [[0, 3, 4, 7], [1, 2, 5, 6]]   # slot si hosts q-block HALF_QBS[h][si]

_CACHE = {}


def _host_tables(half):
    """Per-core mask thresholds [32].

    Mask for slot si, masked-index j (k-block kx = SLOT_NK[si]-8+j):
    valid(i, c) iff qoff + c >= kx*128 + i  iff  (c - i) >= 128*kx - qoff.
    """
    thr = np.zeros(32, dtype=np.float32)
    for si, nk in enumerate(SLOT_NK):
        qoff = HALF_QBS[half][si] * QB
        for j in range(8):
            kx = nk - 8 + j
            thr[si * 8 + j] = 128.0 * kx - float(qoff)
    return thr


def _build_program():
    import concourse.bass as bass
    import concourse.mybir as mybir
    import concourse.tile as tile

    f32 = mybir.dt.float32
    f16 = mybir.dt.float16
    AF = mybir.ActivationFunctionType
    MS = bass.MemorySpace
    nc = bass.Bass("TRN2", target_bir_lowering=True, debug=False,
                   enable_asserts=False)

    def r(ap):
        # float32r view: same bits, 4x faster PE (1 cyc/row at free >= 256)
        return ap.bitcast(mybir.dt.float32r)

    xt_d = nc.dram_tensor("xt", [NE, T], f32, kind="ExternalInput").ap()
    xq_d = nc.dram_tensor("xq", [NE, 4 * QB], f32, kind="ExternalInput").ap()
    wkv_d = nc.dram_tensor("wkv", [NE, 128], f32, kind="ExternalInput").ap()
    wq_d = nc.dram_tensor("wq", [NE, HD], f32, kind="ExternalInput").ap()
    identh_d = nc.dram_tensor("identh", [128, 64], f32, kind="ExternalInput").ap()
    dtab_d = nc.dram_tensor("dtab", [128, QB], f16, kind="ExternalInput").ap()
    thr_d = nc.dram_tensor("thr", [128, 32], f32, kind="ExternalInput").ap()
    out_d = nc.dram_tensor("out", [4 * QB, HD], f32, kind="ExternalOutput").ap()

    with tile.TileContext(nc) as tc:
        with (
            tc.tile_pool(name="consts", bufs=1) as cpool,
            tc.tile_pool(name="big", bufs=1) as bigpool,
            tc.tile_pool(name="xt", bufs=2) as xtpool,
            tc.tile_pool(name="pt", bufs=3) as ptpool,
            tc.tile_pool(name="rec", bufs=2) as recpool,
            tc.tile_pool(name="onat", bufs=2) as onatpool,
            tc.tile_pool(name="sps", bufs=2, space=MS.PSUM) as spool,
            tc.tile_pool(name="ops", bufs=2, space=MS.PSUM) as opool,
            tc.tile_pool(name="projps", bufs=2, space=MS.PSUM) as projpool,
        ):
            # ---- constants (split across Pool + SP DMA queues) ----
            wkv_sb = cpool.tile([128, NT, 128], f32)
            nc.gpsimd.dma_start(wkv_sb[:], wkv_d.rearrange("(nt p) m -> p nt m", p=128))
            wq_sb = cpool.tile([128, NT, HD], f32)
            nc.sync.dma_start(wq_sb[:], wq_d.rearrange("(nt p) m -> p nt m", p=128))
            identh = cpool.tile([128, 64], f32)
            nc.sync.dma_start(identh[:], identh_d[:])
            dtab = cpool.tile([128, QB], f16)
            nc.sync.dma_start(dtab[:], dtab_d[:])
            thr = cpool.tile([128, 32], f32)
            nc.sync.dma_start(thr[:], thr_d[:])

            # ---- persistent sbuf state ----
            kvt = bigpool.tile([128, T], f32)          # 0:64 K^T, 64:128 V^T
            qt_sel = bigpool.tile([64, 4 * QB], f32)   # slot-ordered Q^T
            xq_sb = bigpool.tile([128, NT, 4, QB], f32)
            v_aug = bigpool.tile([128, 32, 65], f16)   # V natural + ones col
            nc.vector.memset(v_aug[:, :, 64:65], 1.0)

            def emit_attention(si):
                nk = SLOT_NK[si]
                npair = nk // 2
                o_ps = opool.tile([128, 4, 65], f32, tag="ops")
                pts = [None] * npair

                def emit_pv(p):
                    # start=True pends-to-zero the WHOLE 2KB psum bank, so only
                    # the first matmul of the bank may set it; the other kx==0
                    # writes land on pending-zero bytes (overwrite semantics).
                    pt = pts[p]
                    for half_i, kx in enumerate((2 * p, 2 * p + 1)):
                        for qc in range(4):
                            nc.tensor.matmul(
                                o_ps[:, qc, :],
                                pt[:, half_i, qc * 128:(qc + 1) * 128],
                                v_aug[:, kx, :],
                                start=(kx == 0 and qc == 0),
                                stop=(kx == nk - 1),
                                skip_group_check=True)

                for p in range(npair):
                    ka, kb2 = 2 * p, 2 * p + 1
                    s_ps = spool.tile([128, 2, QB], f32, tag="sps")
                    nc.tensor.matmul(
                        s_ps[:, 0, :],
                        r(kvt[0:64, ka * KB:(ka + 1) * KB]),
                        r(qt_sel[:, si * QB:(si + 1) * QB]),
                        start=True, stop=True)
                    nc.tensor.matmul(
                        s_ps[:, 1, :],
                        r(kvt[0:64, kb2 * KB:(kb2 + 1) * KB]),
                        r(qt_sel[:, si * QB:(si + 1) * QB]),
                        start=True, stop=True)
                    pt = ptpool.tile([128, 2, QB], f16, tag="pt")
                    pts[p] = pt
                    nc.scalar.activation(pt[:], s_ps[:], AF.Exp,
                                         scale=float(HD) ** -0.5)
                    for half_i, kx in enumerate((ka, kb2)):
                        j = kx - (nk - 8)
                        if j >= 0:
                            # pt = (dtab >= thr) * pt ; thr = 128*kx - qoff
                            nc.vector.scalar_tensor_tensor(
                                pt[:, half_i, :],
                                dtab[:],
                                thr[:, si * 8 + j: si * 8 + j + 1],
                                pt[:, half_i, :],
                                mybir.AluOpType.is_ge,
                                mybir.AluOpType.mult)
                    if p >= 2:
                        emit_pv(p - 2)
                emit_pv(npair - 2)
                emit_pv(npair - 1)
                # epilogue: denom reciprocal + scale, natural orientation
                for qc in range(4):
                    rec = recpool.tile([128, 1], f32, tag="rec")
                    nc.vector.reciprocal(rec[:], o_ps[:, qc, 64:65])
                    o_nat = onatpool.tile([128, HD], f32, tag="onat")
                    nc.scalar.activation(o_nat[:], o_ps[:, qc, 0:HD], AF.Copy,
                                         scale=rec[:])
                    nc.sync.dma_start(
                        out_d[si * QB + qc * 128: si * QB + (qc + 1) * 128, :],
                        o_nat[:])

            # ---- main pipeline over t-blocks ----
            for tb in range(NQB):
                xt_sb = xtpool.tile([128, NT, QB], f32, tag="xt")
                nc.gpsimd.dma_start(
                    xt_sb[:, 0:4, :],
                    xt_d[0:NE // 2, tb * QB:(tb + 1) * QB].rearrange(
                        "(nt p) t -> p nt t", p=128))
                nc.sync.dma_start(
                    xt_sb[:, 4:8, :],
                    xt_d[NE // 2:NE, tb * QB:(tb + 1) * QB].rearrange(
                        "(nt p) t -> p nt t", p=128))
                if tb % 2 == 0:
                    si = tb // 2
                    nc.gpsimd.dma_start(
                        xq_sb[:, :, si, :],
                        xq_d[:, si * QB:(si + 1) * QB].rearrange(
                            "(nt p) t -> p nt t", p=128))
                kv_ps = projpool.tile([128, QB], f32, tag="proj")
                for ni in range(NT):
                    nc.tensor.matmul(kv_ps[:], r(wkv_sb[:, ni, :]),
                                     r(xt_sb[:, ni, :]),
                                     start=(ni == 0), stop=(ni == NT - 1))
                nc.vector.tensor_copy(kvt[:, tb * QB:(tb + 1) * QB], kv_ps[:])
                # V natural (fp16) for this t-block's 4 k-blocks
                vt_ps = projpool.tile([128, 4, 64], f32, tag="proj")
                for j in range(QB // KB):
                    kb = tb * (QB // KB) + j
                    nc.tensor.matmul(
                        vt_ps[:, j, :], kvt[64:128, kb * KB:(kb + 1) * KB],
                        identh[64:128, 0:64], is_transpose=True,
                        start=(j == 0), stop=(j == 3),
                        skip_group_check=True)
                nc.vector.tensor_copy(
                    v_aug[:, tb * 4:(tb + 1) * 4, 0:64], vt_ps[:])
                if tb % 2 == 0:
                    si = tb // 2
                    q_ps = projpool.tile([64, QB], f32, tag="proj")
                    for ni in range(NT):
                        nc.tensor.matmul(q_ps[:], r(wq_sb[:, ni, :]),
                                         r(xq_sb[:, ni, si, :]),
                                         start=(ni == 0), stop=(ni == NT - 1))
                    nc.vector.tensor_copy(
                        qt_sel[:, si * QB:(si + 1) * QB], q_ps[:])
                else:
                    emit_attention(tb // 2)

    _legalize_matmul_waits(nc)
    return nc


def _legalize_matmul_waits(nc):
    """walrus' LW template encodes at most one sync-wait; hoist extra waits
    from Matmult instructions onto a preceding PE NoOp (same queue, so
    ordering semantics are identical)."""
    import concourse.mybir as mybir

    for f in nc.m.functions:
        for bb in f.blocks:
            new_insts = []
            for inst in bb.instructions:
                si = inst.sync_info
                if (si is not None and si.on_wait and len(si.on_wait) >= 2):
                    for w in si.on_wait:
                        nop = mybir.InstNoOp(
                            name=nc.get_next_instruction_name(),
                            text_hint="wait_hoist", bass_nofuse=True)
                        nop.engine = inst.engine
                        nop.sync_info = mybir.SyncInfo(
                            on_wait=[w], on_update=[])
                        new_insts.append(nop)
                    inst.sync_info = mybir.SyncInfo(
                        on_wait=[], on_update=list(si.on_update or []))
                new_insts.append(inst)
            del bb.instructions[:]
            for i in new_insts:
                bb.instructions.append(i)


def _make_inputs(x, Wq, Wk, Wv):
    wkv = np.ascontiguousarray(np.concatenate([Wk, Wv], axis=1), dtype=np.float32)
    wq = np.ascontiguousarray(np.asarray(Wq, dtype=np.float32))
    identh = np.zeros((128, 64), dtype=np.float32)
    identh[64:128, :] = np.eye(64, dtype=np.float32)
    col = np.arange(QB, dtype=np.float32)[None, :]
    row = np.arange(128, dtype=np.float32)[:, None]
    dtab = np.ascontiguousarray((col - row).astype(np.float16))  # [128, QB]

    in_maps = []
    for c in range(8):
        b, half = c // 2, c % 2
        thr = _host_tables(half)
        thr_rep = np.ascontiguousarray(np.tile(thr[None, :], (128, 1)))
        xt = np.ascontiguousarray(np.asarray(x[b], dtype=np.float32).T)
        xq = np.ascontiguousarray(np.concatenate(
            [xt[:, qb * QB:(qb + 1) * QB] for qb in HALF_QBS[half]], axis=1))
        in_maps.append({
            "xt": xt, "xq": xq, "wkv": wkv, "wq": wq, "identh": identh,
            "dtab": dtab, "thr": thr_rep,
        })
    return in_maps


def kernel(x, Wq, Wk, Wv, _want_results=False, _trace=False):
    from concourse import bass_utils

    if "prog" not in _CACHE:
        _CACHE["prog"] = _build_program()
    nc = _CACHE["prog"]
    in_maps = _make_inputs(x, Wq, Wk, Wv)
    res = bass_utils.run_bass_kernel_spmd(nc, in_maps, core_ids=list(range(8)),
                                          trace=_trace)
    out = np.zeros((B, T, HD), dtype=np.float32)
    for c in range(8):
        b, half = c // 2, c % 2
        o = res.results[c]["out"]
        for si in range(4):
            qb = HALF_QBS[half][si]
            out[b, qb * QB:(qb + 1) * QB, :] = o[si * QB:(si + 1) * QB, :]
    if _want_results:
        return out, res
    return out
